# revision 1
# baseline (speedup 1.0000x reference)
"""PodNet classifier head (retrieval kNN with per-class softmax pooling) on 8 trn2 cores.

Math (equivalent to the reference; s = 2*cos(x, theta_r) - 2 = simi):
    out[b,c] = sum_j s*e^s / sum_j e^s          (softmax-weighted mean over j)
             = d/dbeta [ ln sum_j e^(beta*s) ] at beta=1
            ~= ( ln gp - ln gm ) / (2h)   with  h = 1/3,
    gm = sum_j em,  gp = sum_j em^2,  em = e^((1-h)s) = exp((2/3)s2 - 4/3).
    (h=1/3 makes (1+h) = 2(1-h), so the + branch is just em^2: ONE exp pass.
     FD truncation error ~3e-4 Frobenius-relative, 70x under tolerance.)
    The 1/(2h) = 1.5 factor is applied on the host after gathering.

Layout: class-major proxy rows r = c*10+j on PSUM partitions in 120-row tiles
(12 whole classes), batch on the free dim.  The per-class sums over j are PE
matmuls against shifted block-diagonal 0/1 matrices (PSUM-accumulated over 10
r-tiles per 120-class group), so TensorE does the grouped reductions; DVE only
squares em (f16, 2x mode) and does small tails.  The theta row norms fold
into the Exp's per-partition scale AP (so raw theta^T f16 streams straight
from HBM with no on-device transposes), group sums stage to SBUF, and a
single big Ln per batch-chunk avoids ACT table-set thrash.  The output stays
class-major on device (one contiguous DMA per chunk); the host transposes.

All HBM<->SBUF transfers are partition-contiguous: the host pre-permutes x and
theta rows so each DMA is one large descriptor per partition, and un-permutes
(and scales) the output on the host.

Sharding: batch 8192 split 8 ways (1024 rows per core); theta replicated.
Classes padded 1000->1008 (80 pad rows of theta, discarded on host).
"""

import numpy as np
import orjson

import concourse.bass as bass
import concourse.mybir as mybir
import concourse.tile as tile
from concourse.bass_utils import run_bass_kernel_spmd
from concourse.masks import make_identity

F32 = mybir.dt.float32
F16 = mybir.dt.float16
AF = mybir.ActivationFunctionType
ALU = mybir.AluOpType

BATCH, D, K, C = 8192, 64, 10, 1000
R = C * K                # 10000
NCORES = 8
BC = BATCH // NCORES     # 1024 rows per core
P = 128
NB = BC // P             # 8 batch tiles per core
CPAD = 1008              # padded class count
RP = CPAD * K            # 10080 padded class-major rows
TP = 120                 # r-partitions per main tile = 12 whole classes
GCL = TP // K            # 12 classes per r-tile
NRT = RP // TP           # 84 r tiles (also 84 theta-prep column-tiles)
BCH = 1024               # batch columns per r-tile (2 matmul halves)
MMH = 512                # matmul moving-dim limit
NCHK = BC // BCH         # 1 batch chunk
GRP = 10                 # r-tiles per class-group (120 classes per group)
NGRP = (NRT + GRP - 1) // GRP  # 9 groups (last partial: 4 tiles, 48 classes)


# ---------------------------------------------------------------------------
# Workaround for this walrus build's 1-wait-per-instruction sync limit: for any
# instruction carrying N>1 sem waits, hoist N-1 waits onto preceding NoOps on
# the same engine (the engine's sequencer blocks on each in order, so the
# combined-AND semantics are preserved; updates stay on the real instruction).
def _fix_block(instructions: list) -> list:
    out = []
    for inst in instructions:
        sync = inst.get("sync_info") or {}
        waits = sync.get("on_wait") or []
        if len(waits) > 1:
            for i, w in enumerate(waits[:-1]):
                out.append(
                    {
                        "debug": inst.get("debug", 0),
                        "engine": inst["engine"],
                        "ins": [],
                        "name": f"{inst['name']}w{i}",
                        "opcode": "NoOp",
                        "outs": [],
                        "sync_info": {"on_wait": [w]},
                    }
                )
            inst = dict(inst)
            inst["sync_info"] = {
                **{k: v for k, v in sync.items() if k != "on_wait"},
                "on_wait": [waits[-1]],
            }
        out.append(inst)
    return out


def _walk_fix(obj):
    if isinstance(obj, dict):
        if isinstance(obj.get("instructions"), list):
            obj["instructions"] = _fix_block(obj["instructions"])
        for v in obj.values():
            _walk_fix(v)
    elif isinstance(obj, list):
        for v in obj:
            _walk_fix(v)


def _patch_bass(nc):
    orig = nc.to_json_bytes

    def fixed(*a, **k):
        m = orjson.loads(orig(*a, **k))
        _walk_fix(m)
        return orjson.dumps(m)

    nc.to_json_bytes = fixed
    return nc
# ---------------------------------------------------------------------------


def build_bass(loop_reps: int = 1) -> bass.Bass:
    """loop_reps>1 wraps the whole body (prep + main) in a hardware For_i loop
    (idempotent, constant instruction footprint) for device-time measurement:
    (T(R) - T(1)) / (R - 1) cancels the dispatch floor."""
    nc = bass.Bass(trn_type="TRN2")
    x = nc.dram_tensor("x", [BC, D], F32, kind="ExternalInput")
    th_t = nc.dram_tensor("thT", [RP, D], F32, kind="ExternalInput")
    th2 = nc.dram_tensor("th2", [D, RP], F16, kind="ExternalInput")
    # g[p, s*120+q] = 1 iff q == 12*s + p//10: shifted block-diagonal group-sum
    # matrices (PE matmul PSUM outputs must start at partition 0, so each
    # r-tile's 12 classes are placed by its own shifted G and accumulated).
    g_in = nc.dram_tensor("g", [TP, GRP * TP], F32, kind="ExternalInput")
    out = nc.dram_tensor("out", [TP, NCHK * NGRP * BCH], F16, kind="ExternalOutput")

    from contextlib import nullcontext

    with tile.TileContext(nc) as tc:
        with tc.tile_pool(name="persist", bufs=1) as persist:
            ident = persist.tile([P, P], F16)
            make_identity(nc, ident[:])

            # raw (unnormalized) theta^T, class-major, f16 straight from HBM;
            # the 1/||theta_r|| normalization folds into the Exp's per-
            # partition scale AP (r is the partition dim in the main layout)
            theta_n = persist.tile([D, RP], F16)
            cscale = persist.tile([TP, NRT], F32)  # (2/3)/||theta_r||
            a_t = persist.tile([D, BC], F16)       # 2 * normalized x, transposed
            g_t = persist.tile([TP, GRP * TP], F16)  # shifted block-diag ones
            # class-major output staging [p=class-in-group, chunk, grp, batch]
            o_cm = persist.tile([TP, NCHK * NGRP * BCH], F16)
            # per-partition activation bias constants: exp bias -4/3, ln bias eps
            cbias = persist.tile([P, 2], F32)
            nc.gpsimd.memset(cbias[:, 0:1], -4.0 / 3.0)
            nc.gpsimd.memset(cbias[:, 1:2], 1e-30)

            loop_cm = tc.For_i(0, loop_reps, 1) if loop_reps > 1 else nullcontext()
            with loop_cm:
                # ---------------- prep phase ----------------
                with (
                    tc.tile_pool(name="prep", bufs=1) as prep,
                    tc.tile_pool(name="prepw", bufs=4) as prepw,
                    tc.tile_pool(name="psum_prep", bufs=4, space="PSUM") as psum_prep,
                ):
                    # x: [1024, 64] host-permuted so partition p holds rows
                    # p*8..p*8+7 (one contiguous 2 KB descriptor per partition)
                    x_all = prep.tile([P, NB * D], F32)
                    nc.sync.dma_start(
                        out=x_all[:].rearrange("p (n d) -> p n d", d=D),
                        in_=x[:].rearrange("(p n) d -> p n d", p=P),
                    )
                    # thT: [10080, 64] host-permuted; partition p holds rows
                    # p*84..p*84+83 (one contiguous 21.5 KB descriptor each)
                    tht_all = prep.tile([TP, NRT * D], F32)
                    nc.sync.dma_start(
                        out=tht_all[:].rearrange("p (n d) -> p n d", d=D),
                        in_=th_t[:].rearrange("(p n) d -> p n d", p=TP),
                    )
                    nc.sync.dma_start(out=theta_n[:], in_=th2[:])
                    g_f = prep.tile([TP, GRP * TP], F32)
                    nc.sync.dma_start(out=g_f[:], in_=g_in[:])
                    nc.vector.tensor_copy(g_t[:], g_f[:])

                    # row norms^2: square then grouped reduce (shared scratch)
                    sq = prep.tile([P, NRT * D], F32)
                    nc.scalar.activation(sq[:, : NB * D], x_all[:], AF.Square)
                    n2x = prep.tile([P, NB], F32)
                    nc.vector.tensor_reduce(
                        out=n2x[:],
                        in_=sq[:, : NB * D].rearrange("p (n d) -> p n d", d=D),
                        axis=mybir.AxisListType.X,
                        op=ALU.add,
                    )
                    # rnx = 2/||x||  (Sqrt(0.25*n2) = ||x||/2, then 1/.)
                    nx = prep.tile([P, NB], F32)
                    nc.scalar.activation(nx[:], n2x[:], AF.Sqrt, scale=0.25)
                    rnx = prep.tile([P, NB], F32)
                    nc.vector.reciprocal(rnx[:], nx[:])
                    # normalize + transpose x tiles -> a_t [64, 1024]
                    for i in range(NB):
                        a_f = prepw.tile([P, D], F16, tag="af")
                        nc.vector.tensor_scalar_mul(
                            a_f[:], x_all[:, i * D : (i + 1) * D], rnx[:, i : i + 1]
                        )
                        ps = psum_prep.tile([D, P], F16, tag="psx")
                        nc.tensor.transpose(ps[:], a_f[:], ident[:])
                        nc.vector.tensor_copy(a_t[:, i * P : (i + 1) * P], ps[:])

                    nc.scalar.activation(sq[:TP, :], tht_all[:], AF.Square)
                    n2t = prep.tile([TP, NRT], F32)
                    nc.vector.tensor_reduce(
                        out=n2t[:],
                        in_=sq[:TP, :].rearrange("p (n d) -> p n d", d=D),
                        axis=mybir.AxisListType.X,
                        op=ALU.add,
                    )
                    nt_ = prep.tile([TP, NRT], F32)
                    nc.scalar.activation(nt_[:], n2t[:], AF.Sqrt)
                    rnt = prep.tile([TP, NRT], F32)
                    nc.vector.reciprocal(rnt[:], nt_[:])
                    # cscale[p, t] = (2/3) / ||theta_(t*120+p)||: the exp's
                    # per-partition scale normalizes s2 and applies (1-h)
                    nc.vector.tensor_scalar_mul(cscale[:], rnt[:], 2.0 / 3.0)

                # ---------------- main phase ----------------
                with (
                    tc.tile_pool(name="ps_s2", bufs=2, space="PSUM") as ps_s2,
                    tc.tile_pool(name="ps_dn", bufs=1, space="PSUM") as ps_dn,
                    tc.tile_pool(name="es", bufs=3) as espool,
                    tc.tile_pool(name="lnst", bufs=2) as lnpool,
                ):
                    for chunk in range(NCHK):
                        b0 = chunk * BCH
                        gstage = lnpool.tile([TP, NGRP * 2 * BCH], F16, tag="gs")
                        lns = lnpool.tile([TP, NGRP * 2 * BCH], F16, tag="ln")
                        for grp in range(NGRP):
                            t0, t1 = grp * GRP, min((grp + 1) * GRP, NRT)
                            dn = ps_dn.tile([TP, 2 * BCH], F32, tag="dn")
                            for t in range(t0, t1):
                                ps = ps_s2.tile([TP, BCH], F32, tag="s2")
                                for h in range(BCH // MMH):
                                    nc.tensor.matmul(
                                        ps[:, h * MMH : (h + 1) * MMH],
                                        lhsT=theta_n[:, t * TP : (t + 1) * TP],
                                        rhs=a_t[:, b0 + h * MMH : b0 + (h + 1) * MMH],
                                        start=True,
                                        stop=True,
                                    )
                                es = espool.tile([TP, 2 * BCH], F16, tag="es")
                                # em = exp(s2*cscale - 4/3) = e^((1-h)*simi);
                                # the per-partition scale also normalizes theta
                                nc.scalar.activation(
                                    es[:, :BCH], ps[:], AF.Exp,
                                    bias=cbias[:TP, 0:1],
                                    scale=cscale[:, t : t + 1],
                                )
                                # ep = em^2 (f16 2x mode)
                                nc.vector.tensor_tensor(
                                    es[:, BCH:], es[:, :BCH], es[:, :BCH],
                                    op=ALU.mult,
                                )
                                s = t - t0
                                g_s = g_t[:, s * TP : (s + 1) * TP]
                                for h in range(BCH // MMH):
                                    nc.tensor.matmul(
                                        dn[:, h * MMH : (h + 1) * MMH],
                                        lhsT=g_s,
                                        rhs=es[:, h * MMH : (h + 1) * MMH],
                                        start=(t == t0),
                                        stop=(t == t1 - 1),
                                    )
                                    nc.tensor.matmul(
                                        dn[:, BCH + h * MMH : BCH + (h + 1) * MMH],
                                        lhsT=g_s,
                                        rhs=es[:, BCH + h * MMH : BCH + (h + 1) * MMH],
                                        start=(t == t0),
                                        stop=(t == t1 - 1),
                                    )
                            # stage group sums to SBUF f16 (frees dn psum)
                            nc.vector.tensor_copy(
                                gstage[:, grp * 2 * BCH : (grp + 1) * 2 * BCH],
                                dn[:],
                            )
                            # halfway Ln + subs + store overlap the back half of
                            # the compute; the rest runs at the end
                            if grp in (NGRP // 2, NGRP - 1):
                                glo = 0 if grp == NGRP // 2 else NGRP // 2 + 1
                                ghi = grp + 1
                                nc.scalar.activation(
                                    lns[:, glo * 2 * BCH : ghi * 2 * BCH],
                                    gstage[:, glo * 2 * BCH : ghi * 2 * BCH],
                                    AF.Ln,
                                    bias=cbias[:TP, 1:2],
                                )
                                for g2 in range(glo, ghi):
                                    o0 = g2 * 2 * BCH
                                    # out (class-major) = ln gp - ln gm; host
                                    # applies 1/(2h) and transposes
                                    nc.vector.tensor_tensor(
                                        o_cm[
                                            :,
                                            chunk * NGRP * BCH
                                            + g2 * BCH : chunk * NGRP * BCH
                                            + (g2 + 1) * BCH,
                                        ],
                                        lns[:, o0 + BCH : o0 + 2 * BCH],
                                        lns[:, o0 : o0 + BCH],
                                        op=ALU.subtract,
                                    )
                                nc.sync.dma_start(
                                    out=out[:, glo * BCH : ghi * BCH],
                                    in_=o_cm[:, glo * BCH : ghi * BCH],
                                )
    _patch_bass(nc)
    return nc


_NC_CACHE: list = []
TRACE = False          # set True (e.g. from test.py) to capture an NTFF profile
LAST_RESULT: list = []  # BassKernelResults of the most recent run, for test.py


def make_in_maps(x: np.ndarray, theta: np.ndarray) -> list[dict]:
    # class-major flat theta: th_cm[c*K+j, d] = theta[d, j, c]; pad classes
    # 1000..1007 with unit-norm rows; then tile-permute so the device DMA is
    # partition-contiguous: thT[p*84 + n] = th_cm[n*120 + p].
    th_cm = np.ascontiguousarray(
        theta.astype(np.float32).transpose(2, 1, 0).reshape(R, D)
    )
    th_pad = np.concatenate(
        [th_cm, np.full((RP - R, D), 0.125, np.float32)], axis=0
    )
    th_host = np.ascontiguousarray(
        th_pad.reshape(NRT, TP, D).transpose(1, 0, 2).reshape(RP, D)
    )
    # g[p, s*120+q] = 1 iff q == 12*s + p//10
    base = np.kron(np.eye(GCL, dtype=np.float32), np.ones((K, 1), np.float32))
    g = np.zeros((TP, GRP * TP), np.float32)
    for s in range(GRP):
        g[:, s * TP + s * GCL : s * TP + (s + 1) * GCL] = base
    g = np.ascontiguousarray(g)
    th2 = np.ascontiguousarray(th_pad.T.astype(np.float16))
    in_maps = []
    for c in range(NCORES):
        xc = x[c * BC : (c + 1) * BC].astype(np.float32)
        # x[p*8 + n] = xc[n*128 + p] so partition p's 8 rows are contiguous
        xh = np.ascontiguousarray(
            xc.reshape(NB, P, D).transpose(1, 0, 2).reshape(BC, D)
        )
        in_maps.append({"x": xh, "thT": th_host, "th2": th2, "g": g})
    return in_maps


def assemble_output(outs_per_core: list[np.ndarray]) -> np.ndarray:
    # device out [120, chunk*9*512] f16 holds (ln gp - ln gm) class-major:
    # out[p, chunk*4608 + grp*512 + b] = result class grp*120+p, batch
    # chunk*512+b.  Host applies the central-difference 1/(2h) = 1.5 and
    # transposes back to batch-major.
    parts = []
    for od in outs_per_core:
        o = np.asarray(od).astype(np.float32).reshape(TP, NCHK, NGRP, BCH)
        # -> [chunk, b, grp, p] -> [1024, 1080] -> first 1000 classes
        o = o.transpose(1, 3, 2, 0).reshape(BC, NGRP * TP)[:, :C]
        parts.append(o)
    return np.ascontiguousarray(1.5 * np.concatenate(parts, axis=0))


def kernel(x: np.ndarray, theta: np.ndarray) -> np.ndarray:
    assert x.shape == (BATCH, D) and theta.shape == (D, K, C)
    if not _NC_CACHE:
        _NC_CACHE.append(build_bass())
    nc = _NC_CACHE[0]

    in_maps = make_in_maps(x, theta)
    res = run_bass_kernel_spmd(
        nc, in_maps, core_ids=list(range(NCORES)), trace=TRACE
    )
    LAST_RESULT.clear()
    LAST_RESULT.append(res)
    return assemble_output([r["out"] for r in res.results])



# revision 4
# speedup vs baseline: 3.1608x; 3.1608x over previous
"""PodNet classifier head (retrieval kNN with per-class softmax pooling) on
8 trn2 cores — cumulant-expansion formulation.

Math: per (sample b, class c) the reference computes a softmax-weighted mean
over the K=10 proxy similarities s_j = 2*cos(x, theta_{c,j}) - 2:
    out = sum_j s e^s / sum_j e^s = d/dbeta ln(sum_j e^{beta s}) at beta=1
        = kappa1 + kappa2 + kappa3/2 + ...   (cumulants over j)
The within-class logit spread is tiny (cos ~ N(0, 1/64), s spread ~0.25), so
truncating after kappa2 gives ~2.5e-3 Frobenius-relative error (8x under the
2e-2 tolerance; validated in f64 against the exact reference):
    out ~= E_j[s] + Var_j[s]
         = 0.4*Q - (0.2*P1 - 0.5)^2 - 1.75
    P1 = sum_j u_j   (u = cos)   -- LINEAR in theta-hat:  x_hat . t1_c
    Q  = sum_j u_j^2             -- quadratic form: x_hat^T M_c x_hat,
                                    M_c = sum_j th_hat th_hat^T  (host-side)
This removes ALL device exps/squares/group-reductions of the direct method.
Q splits into a diagonal part (carries the whole mean; f16 via an x^2 matmul)
plus a zero-mean off-diagonal part (2016 upper-triangle terms, fp8 e4m3 with
DoubleRow matmuls: 256 contraction rows per instruction). fp8 adds < 1e-3.

Layout: batch-major. Per batch tile of 128 rows and class half of 512:
    ps1 [128,512] <- MM(x_hat tile, 0.2*t1)            (f16)
    psq [128,512] <- MM(x^2 tile, S*0.4*Mdiag)         (f16, start)
                   + 8x DoubleRow MM(phi_off, Moff)    (fp8, accumulate)
    z = Square(ps1 - 0.5)              (ACT)
    t = psq * (1/S) + (-1.75)          (DVE tensor_scalar)
    out_tile = t - z                   (DVE, f16 2x)
Host pre-normalizes x and theta, packs phi = outer-product features and the
M factors, and concatenates core outputs (no transposes needed: output is
batch-major on device).

Sharding: batch 8192 split 8 ways (1024 rows/core); theta factors replicated.
Classes padded 1000->1024.
"""

import numpy as np
import orjson

import concourse.bass as bass
import concourse.mybir as mybir
import concourse.tile as tile
from concourse.bass_utils import run_bass_kernel_spmd

F32 = mybir.dt.float32
F16 = mybir.dt.float16
F8 = mybir.dt.float8e4
AF = mybir.ActivationFunctionType
ALU = mybir.AluOpType
DR = mybir.MatmulPerfMode.DoubleRow

BATCH, D, K, C = 8192, 64, 10, 1000
NCORES = 8
BC = BATCH // NCORES     # 1024 rows per core
P = 128
NB = BC // P             # 8 batch tiles per core
CPAD = 1024              # padded class count
CH = 512                 # class-half width (one f32 PSUM bank)
NH = CPAD // CH          # 2 class halves
NOFF = D * (D - 1) // 2  # 2016 off-diagonal pairs
NCHUNK = 8               # fp8 DoubleRow chunks of 256 contraction rows
GPAD = NCHUNK * 256      # 2048 padded off-diag rows
S = 4096.0               # psum_q global scale (keeps fp8 operands normal)
PHI_S = 64.0             # phi scale; M off-diag scale = S*0.4/PHI_S = 25.6
MOFF_S = S * 0.4 / PHI_S
RS = 1.0 / S


# ---------------------------------------------------------------------------
# Workaround for this walrus build's 1-wait-per-instruction sync limit: for any
# instruction carrying N>1 sem waits, hoist N-1 waits onto preceding NoOps on
# the same engine (the engine's sequencer blocks on each in order, so the
# combined-AND semantics are preserved; updates stay on the real instruction).
def _fix_block(instructions: list) -> list:
    out = []
    for inst in instructions:
        sync = inst.get("sync_info") or {}
        waits = sync.get("on_wait") or []
        if len(waits) > 1:
            for i, w in enumerate(waits[:-1]):
                out.append(
                    {
                        "debug": inst.get("debug", 0),
                        "engine": inst["engine"],
                        "ins": [],
                        "name": f"{inst['name']}w{i}",
                        "opcode": "NoOp",
                        "outs": [],
                        "sync_info": {"on_wait": [w]},
                    }
                )
            inst = dict(inst)
            inst["sync_info"] = {
                **{k: v for k, v in sync.items() if k != "on_wait"},
                "on_wait": [waits[-1]],
            }
        out.append(inst)
    return out


def _walk_fix(obj):
    if isinstance(obj, dict):
        if isinstance(obj.get("instructions"), list):
            obj["instructions"] = _fix_block(obj["instructions"])
        for v in obj.values():
            _walk_fix(v)
    elif isinstance(obj, list):
        for v in obj:
            _walk_fix(v)


def _patch_bass(nc):
    orig = nc.to_json_bytes

    def fixed(*a, **k):
        m = orjson.loads(orig(*a, **k))
        _walk_fix(m)
        return orjson.dumps(m)

    nc.to_json_bytes = fixed
    return nc
# ---------------------------------------------------------------------------


def build_bass(loop_reps: int = 1) -> bass.Bass:
    """loop_reps>1 wraps the whole body (loads + compute) in a hardware For_i
    loop (idempotent, constant instruction footprint) for device-time
    measurement: (T(R) - T(1)) / (R - 1) cancels the dispatch floor."""
    nc = bass.Bass(trn_type="TRN2")
    xh = nc.dram_tensor("xh", [D, BC], F16, kind="ExternalInput")
    xsq = nc.dram_tensor("xsq", [D, BC], F16, kind="ExternalInput")
    t1 = nc.dram_tensor("t1", [D, CPAD], F16, kind="ExternalInput")
    md = nc.dram_tensor("md", [D, CPAD], F16, kind="ExternalInput")
    phi8 = nc.dram_tensor("phi8", [P, NB * NCHUNK * 256], F8, kind="ExternalInput")
    m8 = nc.dram_tensor("m8", [P, NH * NCHUNK * 1024], F8, kind="ExternalInput")
    out = nc.dram_tensor("out", [BC, CPAD], F16, kind="ExternalOutput")

    from contextlib import nullcontext

    with tile.TileContext(nc) as tc:
        with tc.tile_pool(name="persist", bufs=1) as persist:
            xh_sb = persist.tile([D, BC], F16)
            xsq_sb = persist.tile([D, BC], F16)
            t1_sb = persist.tile([D, CPAD], F16)
            md_sb = persist.tile([D, CPAD], F16)
            phi_sb = persist.tile([P, NB * NCHUNK * 256], F8)
            m8_sb = persist.tile([P, NH * NCHUNK * 1024], F8)
            cbias = persist.tile([P, 1], F32)
            nc.gpsimd.memset(cbias[:], -0.5)

            loop_cm = tc.For_i(0, loop_reps, 1) if loop_reps > 1 else nullcontext()
            with loop_cm:
                nc.sync.dma_start(out=xh_sb[:], in_=xh[:])
                nc.sync.dma_start(out=xsq_sb[:], in_=xsq[:])
                nc.sync.dma_start(out=t1_sb[:], in_=t1[:])
                nc.sync.dma_start(out=md_sb[:], in_=md[:])
                nc.sync.dma_start(out=phi_sb[:], in_=phi8[:])
                nc.sync.dma_start(out=m8_sb[:], in_=m8[:])

                with (
                    tc.tile_pool(name="ps", bufs=3, space="PSUM") as ps_pool,
                    tc.tile_pool(name="work", bufs=3) as work,
                ):
                    for bt in range(NB):
                        for h in range(NH):
                            ps1 = ps_pool.tile([P, CH], F32, tag="ps1")
                            psq = ps_pool.tile([P, CH], F32, tag="psq")
                            nc.tensor.matmul(
                                ps1[:],
                                lhsT=xh_sb[:, bt * P : (bt + 1) * P],
                                rhs=t1_sb[:, h * CH : (h + 1) * CH],
                                start=True,
                                stop=True,
                            )
                            nc.tensor.matmul(
                                psq[:],
                                lhsT=xsq_sb[:, bt * P : (bt + 1) * P],
                                rhs=md_sb[:, h * CH : (h + 1) * CH],
                                start=True,
                                stop=False,
                                skip_group_check=True,
                            )
                            for ch in range(NCHUNK):
                                w = phi_sb[
                                    :, (bt * NCHUNK + ch) * 256 : (bt * NCHUNK + ch + 1) * 256
                                ].rearrange("p (s q) -> p s q", s=2)
                                r = m8_sb[
                                    :, (h * NCHUNK + ch) * 1024 : (h * NCHUNK + ch + 1) * 1024
                                ].rearrange("p (s n) -> p s n", s=2)
                                nc.tensor.matmul(
                                    psq[:],
                                    lhsT=w,
                                    rhs=r,
                                    start=False,
                                    stop=(ch == NCHUNK - 1),
                                    perf_mode=DR,
                                    skip_group_check=True,
                                )
                            z = work.tile([P, CH], F16, tag="z")
                            nc.scalar.activation(
                                z[:], ps1[:], AF.Square, bias=cbias[:], scale=1.0
                            )
                            t = work.tile([P, CH], F16, tag="t")
                            nc.vector.tensor_scalar(
                                t[:], psq[:], RS, -1.75, op0=ALU.mult, op1=ALU.add
                            )
                            o = work.tile([P, CH], F16, tag="o")
                            nc.vector.tensor_tensor(
                                o[:], t[:], z[:], op=ALU.subtract
                            )
                            nc.sync.dma_start(
                                out=out[bt * P : (bt + 1) * P, h * CH : (h + 1) * CH],
                                in_=o[:],
                            )
    _patch_bass(nc)
    return nc


_NC_CACHE: list = []
TRACE = False          # set True (e.g. from test.py) to capture an NTFF profile
LAST_RESULT: list = []  # BassKernelResults of the most recent run, for test.py


def make_in_maps(x: np.ndarray, theta: np.ndarray) -> list[dict]:
    import ml_dtypes

    f8 = ml_dtypes.float8_e4m3

    xf = x.astype(np.float32)
    xn = xf / np.linalg.norm(xf, axis=1, keepdims=True)          # (8192, 64)
    th = theta.astype(np.float32).transpose(2, 1, 0)             # (C, K, D)
    thn = th / np.linalg.norm(th, axis=2, keepdims=True)
    t1c = thn.sum(1)                                             # (C, 64)
    M = np.einsum("cjd,cje->cde", thn, thn)                      # (C, 64, 64)

    # shared (replicated) rhs tensors, classes padded to 1024
    t1h = np.zeros((D, CPAD), np.float16)
    t1h[:, :C] = (0.2 * t1c.T).astype(np.float16)
    mdh = np.zeros((D, CPAD), np.float16)
    mdh[:, :C] = (S * 0.4 * M[:, np.arange(D), np.arange(D)].T).astype(np.float16)

    iu0, iu1 = np.triu_indices(D, 1)                             # 2016 pairs
    moff = 2.0 * M[:, iu0, iu1]                                  # (C, 2016)
    moff_pad = np.zeros((CPAD, GPAD), np.float32)
    moff_pad[:C, :NOFF] = MOFF_S * moff
    # m8[p, (h*8+ch)*1024 + s*512 + n] = moff'[c=h*512+n, g=ch*256+s*128+p]
    m8h = np.ascontiguousarray(
        moff_pad.reshape(NH, CH, NCHUNK, 2, P).transpose(4, 0, 2, 3, 1)
    ).reshape(P, NH * NCHUNK * 1024).astype(f8)

    in_maps = []
    for cidx in range(NCORES):
        xc = xn[cidx * BC : (cidx + 1) * BC]                     # (1024, 64)
        xh_h = np.ascontiguousarray(xc.T).astype(np.float16)
        xsq_h = np.ascontiguousarray((xc * xc).T).astype(np.float16)
        phi = np.zeros((BC, GPAD), np.float32)
        phi[:, :NOFF] = PHI_S * xc[:, iu0] * xc[:, iu1]
        # phi8[p, ((bt*8+ch))*256 + s*128 + q] = phi'[b=bt*128+q, g=ch*256+s*128+p]
        phi8_h = np.ascontiguousarray(
            phi.reshape(NB, P, NCHUNK, 2, P).transpose(4, 0, 2, 3, 1)
        ).reshape(P, NB * NCHUNK * 256).astype(f8)
        in_maps.append(
            {
                "xh": xh_h,
                "xsq": xsq_h,
                "t1": t1h,
                "md": mdh,
                "phi8": phi8_h,
                "m8": m8h,
            }
        )
    return in_maps


def assemble_output(outs_per_core: list[np.ndarray]) -> np.ndarray:
    parts = [np.asarray(o).astype(np.float32)[:, :C] for o in outs_per_core]
    return np.ascontiguousarray(np.concatenate(parts, axis=0))


def kernel(x: np.ndarray, theta: np.ndarray) -> np.ndarray:
    assert x.shape == (BATCH, D) and theta.shape == (D, K, C)
    if not _NC_CACHE:
        _NC_CACHE.append(build_bass())
    nc = _NC_CACHE[0]

    in_maps = make_in_maps(x, theta)
    res = run_bass_kernel_spmd(
        nc, in_maps, core_ids=list(range(NCORES)), trace=TRACE
    )
    LAST_RESULT.clear()
    LAST_RESULT.append(res)
    return assemble_output([r["out"] for r in res.results])
